# revision 14
# baseline (speedup 1.0000x reference)
"""Trainium2 Bass kernel for nn_Block_24111946399747 (dense transformer block).

Strategy (8 NeuronCores, two SPMD launches; heads->rows reshard on host):

Launch 1 (head-sharded attention; core i owns heads 2i, 2i+1):
  - Pipelined per 512-token block: LN stats (DVE bn_stats on f32 x) ->
    LN apply (ACT, f32 -> fp8 e4m3 h) -> blocked DRAM store -> fast
    contiguous DMA transposes (split across sync+scalar HWDGE queues) ->
    fp8 DoubleRow QKV matmuls (256-contraction per pass, weights x64).
  - Attention (bf16): S^T = k^T q per 128-key block, exp fused over
    pairs of blocks on ACT (PSUM 2-bank span), causal mask on DVE,
    y^T = v^T P^T and rowsum via ones-matmul accumulated in PSUM.
    Epilogue per (b, head): batched DVE reciprocal of rowsums, K=1
    ones-matmul broadcast of 1/rowsum across partitions, DVE scale,
    y_t emitted as fp8.

Host: stack per-core y_t -> y_all [2048, 4096] fp8, hand each core its
512-token column slice.

Launch 2 (row-sharded; core i owns token rows 512i..512i+512):
  - proj in fp8 DoubleRow (y pairs stacked via AP rearrange, w_pr x64):
    x2 = y^T w_pr/64 + (x + b_pr), residual kept f32 in SBUF.
  - x2 -> bf16 blocked DRAM -> fast transposes -> x2_t feature-major.
  - MLP in bf16: m = Gelu(w_fc^T x2_t + b_fc) (ACT), out = m^T w_out
    + (x2 + b_out), token-major f32 out; host concatenates.
"""

import math
import os
import sys

import numpy as np

if "/opt/trn_rl_repo" not in sys.path:
    sys.path.insert(0, "/opt/trn_rl_repo")

import ml_dtypes  # noqa: E402

import concourse.bass as bass  # noqa: E402,F401
import concourse.mybir as mybir  # noqa: E402
import concourse.tile as tile  # noqa: E402
from concourse import bacc  # noqa: E402
from concourse.bass_utils import run_bass_kernel_spmd  # noqa: E402

B, T, C, H = 2, 2048, 2048, 16
HD = C // H            # 128 head dim
N_CORES = 8
HPC = H // N_CORES     # 2 heads per core
NTOK = B * T           # 4096 tokens
RPC = NTOK // N_CORES  # 512 rows per core
P = 128
KC = C // P            # 16 contraction chunks over C
KP = C // 256          # 8 fp8-pair chunks over C
F4 = 4 * C             # 8192
MC4 = F4 // P          # 64 contraction chunks over 4C
NT = NTOK // P         # 32 token tiles of 128
NTT = NTOK // 512      # 8 token tiles of 512
JTT = T // 512         # 4 query tiles of 512 per batch
EPS = 1e-6
WS = 64.0              # fp8 weight scale
BF16 = mybir.dt.bfloat16
F8 = mybir.dt.float8e4
F32 = mybir.dt.float32
ALU = mybir.AluOpType
ACT = mybir.ActivationFunctionType
DR = mybir.MatmulPerfMode.DoubleRow

_BUILT1 = None
_BUILT2 = None
# Phase gating for bisection: prefix of "AC" (launch 1) / "DE" (launch 2).
_PHASES = os.environ.get("KERNEL_PHASES", "ACDE")


# ======================= Launch 1: LN + QKV + attention =======================

def _emit_attn(nc, tc, io):
    x_full = io["x_full"]
    w_qk, b_qk, w_v, b_v = io["w_qk"], io["b_qk"], io["w_v"], io["b_v"]
    cmask, yt_out = io["cmask"], io["yt"]
    ph = _PHASES

    from contextlib import ExitStack

    with ExitStack() as es:
        constp = es.enter_context(tc.tile_pool(name="constp", bufs=1))
        dramp = es.enter_context(tc.tile_pool(name="dramp", bufs=1,
                                              space="DRAM"))
        ones_sb = constp.tile([P, P], BF16, name="ones_sb")
        nc.any.memset(ones_sb[:], 1.0)
        eps_sb = constp.tile([P, 1], F32, name="eps_sb")
        nc.any.memset(eps_sb[:], EPS)
        mask_sb = constp.tile([P, P], BF16, name="mask_sb")
        nc.sync.dma_start(mask_sb[:], cmask[:, :])
        b_qk_sb = constp.tile([P, 4], F32, name="b_qk_sb")
        nc.sync.dma_start(b_qk_sb[:], b_qk.ap().rearrange("(c p) -> p c", p=P))
        b_v_sb = constp.tile([P, HPC], F32, name="b_v_sb")
        nc.sync.dma_start(b_v_sb[:], b_v.ap().rearrange("(c p) -> p c", p=P))

        # persistent QKV outputs (allocated before transient pools)
        persbc = es.enter_context(tc.tile_pool(name="persbc", bufs=1))
        qk_t = persbc.tile([P, 4, NTOK], BF16, name="qk_t")
        v_sb = persbc.tile([P, NT, HPC * HD], BF16, name="v_sb")
        wqp = es.enter_context(tc.tile_pool(name="wqp", bufs=1))
        w_qk_sb = wqp.tile([P, KP, 2, 4 * P], F8, name="w_qk_sb")
        nc.sync.dma_start(
            w_qk_sb[:], w_qk.ap().rearrange("ko p two f -> p ko two f"))
        w_v_sb = wqp.tile([P, KP, 2, HPC * HD], F8, name="w_v_sb")
        nc.sync.dma_start(
            w_v_sb[:], w_v.ap().rearrange("ko p two f -> p ko two f"))

        # blocked h storage: per 512-token block, KP chunks of [512, 128]
        # u16 pairs (contiguous transpose sources)
        h_blks = [dramp.tile([KP, 512, P], BF16, name=f"h_blk{g}")
                  for g in range(NTT)]
        # blocked v storage: per block, [fc][t4] chunks of [128 feat, 128 tok]
        v_blks = [dramp.tile([HPC, 4, P, P], BF16, name=f"v_blk{g}")
                  for g in range(NTT)]

        # ---------------- Phase A: LN + QKV, pipelined per block ----------
        GRP = 4
        inv_ws = 1.0 / WS
        with tc.tile_pool(name="lnp", bufs=GRP + 2) as lnp, \
             tc.tile_pool(name="lnw", bufs=3) as lnw, \
             tc.tile_pool(name="lns", bufs=2) as lns, \
             tc.tile_pool(name="hfp", bufs=3) as hfp, \
             tc.tile_pool(name="htp", bufs=2) as htp, \
             tc.tile_pool(name="vtp", bufs=2) as vtp, \
             tc.tile_pool(name="qps", bufs=2, space="PSUM") as qps, \
             tc.tile_pool(name="vps", bufs=2, space="PSUM") as vps:
            for g in range(NTT):
                # --- LN for the 4 token tiles of this block ---
                xts = []
                mvg = lns.tile([P, GRP, 2], F32, tag="mvg")
                for j in range(GRP):
                    t = g * GRP + j
                    xt = lnp.tile([P, C], F32, tag="xt")
                    nc.sync.dma_start(xt[:], x_full[t * P:(t + 1) * P, :])
                    xts.append(xt)
                    stats = lnw.tile([P, 4, 6], F32, tag="stats")
                    xr = xt[:].rearrange("p (s f) -> p s f", f=512)
                    for s in range(4):
                        nc.vector.bn_stats(stats[:, s, :], xr[:, s, :])
                    nc.vector.bn_aggr(mvg[:, j, :], stats[:])
                stdg = lns.tile([P, GRP], F32, tag="stdg")
                nc.scalar.activation(stdg[:], mvg[:, :, 1], ACT.Sqrt,
                                     bias=eps_sb[:])
                rstdg = lns.tile([P, GRP], F32, tag="rstdg")
                nc.vector.reciprocal(rstdg[:], stdg[:])
                nmrg = lns.tile([P, GRP], F32, tag="nmrg")
                nc.vector.tensor_mul(nmrg[:], mvg[:, :, 0], rstdg[:])
                nc.vector.tensor_scalar_mul(nmrg[:], nmrg[:], -1.0)
                for j in range(GRP):
                    ht = hfp.tile([P, C], F8, tag="ht")
                    nc.scalar.activation(ht[:], xts[j][:], ACT.Identity,
                                         bias=nmrg[:, j:j + 1],
                                         scale=rstdg[:, j:j + 1])
                    # store as u16 pairs into blocked DRAM
                    nc.sync.dma_start(
                        h_blks[g][:, j * P:(j + 1) * P, :]
                        .rearrange("c t f -> t c f"),
                        ht[:].bitcast(BF16).rearrange("p (c f) -> p c f", f=P))
                # --- transposes (split across the two HWDGE queues) ---
                h_t = htp.tile([P, KP, 512], BF16, tag="h_t")
                for ko in range(KP):
                    nc.sync.dma_start_transpose(h_t[:, ko, :],
                                                h_blks[g][ko, :, :])
                # fp8 pair view: [p][ko][two][tok]
                hp = (h_t[:].bitcast(F8)
                      .rearrange("p c (t two) -> p c two t", two=2))
                # --- QKV DoubleRow matmuls ---
                for fc in range(4):  # q0,q1,k0,k1 feature chunks
                    ps = qps.tile([P, 512], F32, tag="qk_ps")
                    for ko in range(KP):
                        nc.tensor.matmul(
                            ps[:], w_qk_sb[:, ko, :, fc * P:(fc + 1) * P],
                            hp[:, ko], start=(ko == 0), stop=(ko == KP - 1),
                            perf_mode=DR)
                    nc.scalar.activation(
                        qk_t[:, fc, g * 512:(g + 1) * 512], ps[:],
                        ACT.Identity, bias=b_qk_sb[:, fc:fc + 1],
                        scale=inv_ws)
                # v feature-major via DR, then blocked store + transpose
                v_td = vtp.tile([P, HPC, 512], BF16, tag="v_td")
                for fc in range(HPC):
                    psv = vps.tile([P, 512], F32, tag="v_ps")
                    for ko in range(KP):
                        nc.tensor.matmul(
                            psv[:], w_v_sb[:, ko, :, fc * P:(fc + 1) * P],
                            hp[:, ko],
                            start=(ko == 0), stop=(ko == KP - 1),
                            perf_mode=DR)
                    nc.scalar.activation(
                        v_td[:, fc, :], psv[:], ACT.Identity,
                        bias=b_v_sb[:, fc:fc + 1], scale=inv_ws)
                    nc.sync.dma_start(
                        v_blks[g][fc].rearrange("t f k -> f t k"),
                        v_td[:, fc, :].rearrange("p (t k) -> p t k", k=P))
                for fc in range(HPC):
                    for t4 in range(4):
                        nc.sync.dma_start_transpose(
                            v_sb[:, g * 4 + t4, fc * P:(fc + 1) * P],
                            v_blks[g][fc, t4])

        if "C" not in ph:  # dump q_t head 0 into yt and stop
            with tc.tile_pool(name="dmp", bufs=2) as dmp:
                for rb in range(2):
                    t = dmp.tile([P, NTOK], F8, tag="t")
                    nc.vector.tensor_copy(t[:], qk_t[:, rb, :])
                    nc.sync.dma_start(yt_out[rb * P:(rb + 1) * P, :], t[:])
            return

        # ---------------- Phase C: causal attention ----------------
        with tc.tile_pool(name="sps", bufs=2, space="PSUM") as sps, \
             tc.tile_pool(name="yps", bufs=2, space="PSUM") as yps, \
             tc.tile_pool(name="rps", bufs=2, space="PSUM") as rps, \
             tc.tile_pool(name="attp", bufs=2) as attp, \
             tc.tile_pool(name="rvp", bufs=2) as rvp, \
             tc.tile_pool(name="yfp", bufs=3) as yfp:
            inv_sqrt_hd = 1.0 / math.sqrt(HD)
            for b in range(B):
                for hl in range(HPC):
                    q_sl = qk_t[:, hl, b * T:(b + 1) * T]
                    k_sl = qk_t[:, 2 + hl, b * T:(b + 1) * T]
                    for jt in range(JTT):
                        nblk = 4 * (jt + 1)
                        pt = attp.tile([P, 16, 512], BF16, tag="pt")
                        y_ps = yps.tile([P, 512], F32, tag="y_ps")
                        # all-ones stationary: every partition of rs_ps
                        # receives the rowsum (broadcast inside the matmul)
                        rs_ps = rps.tile([P, 512], F32, tag="rs_ps")
                        for ib2 in range(nblk // 2):
                            sp = sps.tile([P, 1024], F32, tag="s_ps")
                            c0s = []
                            for u in range(2):
                                ib = 2 * ib2 + u
                                c0 = max(0, ib * P - jt * 512)
                                c0s.append(c0)
                                nc.tensor.matmul(
                                    sp[:, u * 512 + c0:(u + 1) * 512],
                                    k_sl[:, ib * P:(ib + 1) * P],
                                    q_sl[:, jt * 512 + c0:(jt + 1) * 512],
                                    start=True, stop=True)
                            c0a = c0s[0]
                            pt2 = pt[:, 2 * ib2:2 * ib2 + 2, :].rearrange(
                                "p a b -> p (a b)")
                            nc.scalar.activation(
                                pt2[:, c0a:1024], sp[:, c0a:1024],
                                ACT.Exp, scale=inv_sqrt_hd)
                            for u in range(2):
                                ib = 2 * ib2 + u
                                c0 = c0s[u]
                                if ib >= 4 * jt:  # diagonal 128x128 sub-block
                                    nc.vector.tensor_mul(
                                        pt[:, ib, c0:c0 + P],
                                        pt[:, ib, c0:c0 + P], mask_sb[:])
                                vv = v_sb[:, b * (T // P) + ib,
                                          hl * HD:(hl + 1) * HD]
                                nc.tensor.matmul(
                                    y_ps[:, c0:512], vv, pt[:, ib, c0:512],
                                    start=(ib == 0), stop=(ib == nblk - 1))
                                nc.tensor.matmul(
                                    rs_ps[:, c0:512], ones_sb[:],
                                    pt[:, ib, c0:512],
                                    start=(ib == 0), stop=(ib == nblk - 1))
                        rinv = rvp.tile([P, 512], F32, tag="rinv")
                        nc.vector.reciprocal_approx_fast(rinv[:], rs_ps[:])
                        yf = yfp.tile([P, 512], F8, tag="yf")
                        nc.vector.tensor_mul(yf[:], y_ps[:], rinv[:])
                        nc.sync.dma_start(
                            yt_out[hl * HD:(hl + 1) * HD,
                                   b * T + jt * 512:b * T + (jt + 1) * 512],
                            yf[:])


def _build_attn():
    nc = bacc.Bacc("TRN2", target_bir_lowering=False, debug=False,
                   num_devices=N_CORES)
    io = {}
    io["x_full"] = nc.dram_tensor("x_full", [NTOK, C], F32,
                                  kind="ExternalInput").ap()
    io["w_qk"] = nc.dram_tensor("w_qk", [KP, P, 2, 4 * P], F8,
                                kind="ExternalInput")
    io["b_qk"] = nc.dram_tensor("b_qk", [4 * P], F32, kind="ExternalInput")
    io["w_v"] = nc.dram_tensor("w_v", [KP, P, 2, HPC * HD], F8,
                               kind="ExternalInput")
    io["b_v"] = nc.dram_tensor("b_v", [HPC * HD], F32, kind="ExternalInput")
    io["cmask"] = nc.dram_tensor("cmask", [P, P], BF16,
                                 kind="ExternalInput").ap()
    io["yt"] = nc.dram_tensor("yt", [HPC * HD, NTOK], F8,
                              kind="ExternalOutput").ap()
    with tile.TileContext(nc) as tc:
        _emit_attn(nc, tc, io)
    nc.compile()
    return nc


# ======================= Launch 2: proj + MLP =======================

def _emit_mlp(nc, tc, io):
    y_t_in, x_rows = io["y_t"], io["x_rows"]
    w_pr, b_pr = io["w_pr"], io["b_pr"]
    w_fc, b_fc, w_out, b_out = io["w_fc"], io["b_fc"], io["w_out"], io["b_out"]
    out = io["out"]
    ph = _PHASES
    inv_ws = 1.0 / WS

    from contextlib import ExitStack

    with ExitStack() as es:
        constp = es.enter_context(tc.tile_pool(name="constp", bufs=1))
        dramp = es.enter_context(tc.tile_pool(name="dramp", bufs=1,
                                              space="DRAM"))
        b_fc_sb = constp.tile([P, MC4], F32, name="b_fc_sb")
        nc.sync.dma_start(b_fc_sb[:], b_fc.ap().rearrange("(c p) -> p c", p=P))
        b_pr_sb = constp.tile([P, C], F32, name="b_pr_sb")
        nc.scalar.dma_start(b_pr_sb[:], b_pr.ap()[None, :].to_broadcast((P, C)))
        b_out_sb = constp.tile([P, C], F32, name="b_out_sb")
        nc.scalar.dma_start(b_out_sb[:], b_out.ap()[None, :].to_broadcast((P, C)))

        # blocked x2 storage: KC chunks of [512, 128] bf16
        x2_blk = dramp.tile([KC, 512, P], BF16, name="x2_blk")

        persde = es.enter_context(tc.tile_pool(name="persde", bufs=1))
        x2_f32 = persde.tile([P, 4, C], F32, name="x2_f32")
        x2_t = persde.tile([P, KC, RPC], BF16, name="x2_t")
        y_sb = persde.tile([P, KP, 2, RPC], F8, name="y_sb")

        # ---------------- Phase D: proj + residual ----------------
        with tc.tile_pool(name="pdp", bufs=3) as pdp, \
             tc.tile_pool(name="wprp", bufs=2) as wprp, \
             tc.tile_pool(name="dps", bufs=3, space="PSUM") as dps:
            y_re = y_t_in.ap().rearrange("(ko i p) r -> p ko i r", p=P, i=2)
            for ko in range(KP):
                nc.sync.dma_start(y_sb[:, ko], y_re[:, ko])
            for rb in range(4):
                nc.scalar.dma_start(
                    x2_f32[:, rb, :], x_rows[rb * P:(rb + 1) * P, :])
            for rb in range(4):
                nc.vector.tensor_add(
                    x2_f32[:, rb, :], x2_f32[:, rb, :], b_pr_sb[:])
            for ct in range(4):
                wt = wprp.tile([P, KP, 2, 512], F8, tag="wpr")
                nc.sync.dma_start(
                    wt[:], w_pr.ap()[ct].rearrange("ko p two f -> p ko two f"))
                for rb in range(4):
                    ps = dps.tile([P, 512], F32, tag="pr_ps")
                    for ko in range(KP):
                        nc.tensor.matmul(
                            ps[:], y_sb[:, ko, :, rb * P:(rb + 1) * P],
                            wt[:, ko],
                            start=(ko == 0), stop=(ko == KP - 1),
                            perf_mode=DR)
                    sl = x2_f32[:, rb, ct * 512:(ct + 1) * 512]
                    nc.vector.scalar_tensor_tensor(
                        sl, ps[:], inv_ws, sl, op0=ALU.mult, op1=ALU.add)
                    x2b = pdp.tile([P, 512], BF16, tag="x2b")
                    nc.vector.tensor_copy(x2b[:], sl)
                    nc.sync.dma_start(
                        x2_blk[ct * 4:(ct + 1) * 4, rb * P:(rb + 1) * P, :]
                        .rearrange("c t f -> t c f"),
                        x2b[:].rearrange("p (c f) -> p c f", f=P))
                # transpose-load this ct's four feature chunks immediately
                for kk in range(4):
                    nc.sync.dma_start_transpose(
                        x2_t[:, 4 * ct + kk, :], x2_blk[4 * ct + kk, :, :])

        if "E" not in ph:  # dump x2 and stop
            with tc.tile_pool(name="dmp", bufs=2) as dmp:
                for rb in range(4):
                    nc.sync.dma_start(
                        out[rb * P:(rb + 1) * P, :], x2_f32[:, rb, :])
            return

        # pre-add b_out into the residual (after proj phase)
        for rb in range(4):
            nc.vector.tensor_add(
                x2_f32[:, rb, :], x2_f32[:, rb, :], b_out_sb[:])

        # ---------------- Phase E: MLP + residual ----------------
        with tc.tile_pool(name="mep", bufs=1) as mep, \
             tc.tile_pool(name="wfcp", bufs=4) as wfcp, \
             tc.tile_pool(name="wop", bufs=8) as wop, \
             tc.tile_pool(name="ofp", bufs=3) as ofp, \
             tc.tile_pool(name="eps1", bufs=3, space="PSUM") as eps1, \
             tc.tile_pool(name="eps2", bufs=1, space="PSUM") as eps2:
            m_sb = mep.tile([P, MC4, RPC], BF16, name="m_sb")
            for mc in range(MC4):
                wfc_t = wfcp.tile([P, KC, P], BF16, tag="wfc")
                nc.sync.dma_start(
                    wfc_t[:],
                    w_fc.ap()[:, mc * P:(mc + 1) * P]
                    .rearrange("(ko p) m -> p ko m", p=P))
                ps = eps1.tile([P, 512], F32, tag="fc_ps")
                for ko in range(KC):
                    nc.tensor.matmul(
                        ps[:], wfc_t[:, ko, :], x2_t[:, ko, :],
                        start=(ko == 0), stop=(ko == KC - 1))
                nc.scalar.activation(
                    m_sb[:, mc, :], ps[:], ACT.Gelu,
                    bias=b_fc_sb[:, mc:mc + 1], scale=1.0)
            if "F" in ph:  # debug: dump m chunks 0..15 feature-major
                with tc.tile_pool(name="dmp2", bufs=2) as dmp2:
                    for mc in range(16):
                        mf = dmp2.tile([P, RPC], F32, tag="mf")
                        nc.vector.tensor_copy(mf[:], m_sb[:, mc, :])
                        nc.sync.dma_start(
                            out[(mc % 4) * P:(mc % 4 + 1) * P,
                                (mc // 4) * 512:(mc // 4 + 1) * 512],
                            mf[:])
                return
            for ct in range(4):
                pss = [eps2.tile([P, 512], F32, tag=f"o_ps{rb}",
                                 name=f"o_ps{rb}_{ct}")
                       for rb in range(4)]
                for ko in range(MC4):
                    wo_t = wop.tile([P, 512], BF16, tag="wo")
                    nc.sync.dma_start(
                        wo_t[:],
                        w_out.ap()[ko * P:(ko + 1) * P,
                                   ct * 512:(ct + 1) * 512])
                    for rb in range(4):
                        nc.tensor.matmul(
                            pss[rb][:], m_sb[:, ko, rb * P:(rb + 1) * P],
                            wo_t[:],
                            start=(ko == 0), stop=(ko == MC4 - 1))
                for rb in range(4):
                    of = ofp.tile([P, 512], F32, tag="of")
                    nc.vector.tensor_add(
                        of[:], pss[rb][:],
                        x2_f32[:, rb, ct * 512:(ct + 1) * 512])
                    nc.scalar.dma_start(
                        out[rb * P:(rb + 1) * P, ct * 512:(ct + 1) * 512],
                        of[:])


def _build_mlp():
    nc = bacc.Bacc("TRN2", target_bir_lowering=False, debug=False,
                   num_devices=N_CORES)
    io = {}
    io["y_t"] = nc.dram_tensor("y_t", [C, RPC], F8, kind="ExternalInput")
    io["x_rows"] = nc.dram_tensor("x_rows", [RPC, C], F32,
                                  kind="ExternalInput").ap()
    io["w_pr"] = nc.dram_tensor("w_pr", [4, KP, P, 2, 512], F8,
                                kind="ExternalInput")
    io["b_pr"] = nc.dram_tensor("b_pr", [C], F32, kind="ExternalInput")
    io["w_fc"] = nc.dram_tensor("w_fc", [C, F4], BF16, kind="ExternalInput")
    io["b_fc"] = nc.dram_tensor("b_fc", [F4], F32, kind="ExternalInput")
    io["w_out"] = nc.dram_tensor("w_out", [F4, C], BF16, kind="ExternalInput")
    io["b_out"] = nc.dram_tensor("b_out", [C], F32, kind="ExternalInput")
    io["out"] = nc.dram_tensor("out", [RPC, C], F32,
                               kind="ExternalOutput").ap()
    with tile.TileContext(nc) as tc:
        _emit_mlp(nc, tc, io)
    nc.compile()
    return nc


def _get_built():
    global _BUILT1, _BUILT2
    if _BUILT1 is None:
        _BUILT1 = _build_attn()
    if _BUILT2 is None and any(p in _PHASES for p in "DE"):
        _BUILT2 = _build_mlp()
    return _BUILT1, _BUILT2


# ======================= Host orchestration =======================

def _pack_pairs_interleaved(w, scale):
    """[C, F] -> [KP, 128, 2, F] fp8, pair rows (256k+2j, 256k+2j+1)."""
    f8 = ml_dtypes.float8_e4m3
    wf = np.asarray(w, np.float64) * scale
    Cd, F = wf.shape
    wr = wf.reshape(Cd // 256, 128, 2, F)  # [ko][j][i] = row 256ko+2j+i
    return np.ascontiguousarray(wr.astype(np.float32)).astype(f8)


def _pack_pairs_stacked(w, scale):
    """[C, F] -> [KP, 128, 2, F] fp8, pair rows (256k+j, 256k+128+j)."""
    f8 = ml_dtypes.float8_e4m3
    wf = np.asarray(w, np.float64) * scale
    Cd, F = wf.shape
    wr = wf.reshape(Cd // 256, 2, 128, F).transpose(0, 2, 1, 3)
    return np.ascontiguousarray(wr.astype(np.float32)).astype(f8)


def _prep(x, ln_scale, ln_bias, w_qkv, b_qkv, w_proj, b_proj,
          w_fc, b_fc, w_out, b_out):
    bf = ml_dtypes.bfloat16
    xf = np.ascontiguousarray(np.asarray(x, np.float32).reshape(NTOK, C))
    # Fold LN affine into the QKV projection (exact, in float64).
    w64 = np.asarray(w_qkv, np.float64)
    g = np.asarray(ln_scale, np.float64)
    beta = np.asarray(ln_bias, np.float64)
    w_eff = g[:, None] * w64
    b_eff = np.asarray(b_qkv, np.float64) + beta @ w64

    wq, wk, wv = w_eff[:, :C], w_eff[:, C:2 * C], w_eff[:, 2 * C:]
    bq, bk, bv = b_eff[:C], b_eff[C:2 * C], b_eff[2 * C:]
    cmask = np.triu(np.ones((P, P), np.float32)).astype(bf)

    in1 = []
    for i in range(N_CORES):
        hs = slice(i * HPC * HD, (i + 1) * HPC * HD)
        w_qk_i = _pack_pairs_interleaved(
            np.concatenate([wq[:, hs], wk[:, hs]], axis=1), WS)
        b_qk_i = np.ascontiguousarray(
            np.concatenate([bq[hs], bk[hs]]).astype(np.float32))
        w_v_i = _pack_pairs_interleaved(wv[:, hs], WS)
        b_v_i = np.ascontiguousarray(bv[hs].astype(np.float32))
        in1.append({
            "x_full": xf,
            "w_qk": w_qk_i, "b_qk": b_qk_i, "w_v": w_v_i, "b_v": b_v_i,
            "cmask": cmask,
        })

    # proj weights: [4ct][KP, 128, 2, 512] fp8, stacked pairing
    wp = _pack_pairs_stacked(np.asarray(w_proj, np.float64), WS)  # [8,128,2,2048]
    w_pr_b = np.ascontiguousarray(
        wp.reshape(KP, P, 2, 4, 512).transpose(3, 0, 1, 2, 4))
    w_fc_b = np.asarray(w_fc, np.float32).astype(bf)
    w_out_b = np.asarray(w_out, np.float32).astype(bf)
    b_pr_f = np.ascontiguousarray(np.asarray(b_proj, np.float32))
    b_fc_f = np.ascontiguousarray(np.asarray(b_fc, np.float32))
    b_out_f = np.ascontiguousarray(np.asarray(b_out, np.float32))
    in2_common = {
        "w_pr": w_pr_b, "b_pr": b_pr_f, "w_fc": w_fc_b, "b_fc": b_fc_f,
        "w_out": w_out_b, "b_out": b_out_f,
    }
    return xf, in1, in2_common


def run(inputs, trace=False, trace_cores=None):
    """Run both SPMD launches. Returns (output [B,T,C] f32, res1, res2)."""
    nc1, nc2 = _get_built()
    xf, in1, in2_common = _prep(**inputs)
    kwargs = {}
    if trace:
        kwargs = dict(trace=True,
                      trace_cores=trace_cores if trace_cores else [0])
    res1 = run_bass_kernel_spmd(nc1, in1, core_ids=list(range(N_CORES)),
                                **kwargs)
    y_all = np.concatenate(
        [np.asarray(res1.results[i]["yt"]) for i in range(N_CORES)], axis=0)
    if nc2 is None:
        return y_all, res1, None

    in2 = []
    for i in range(N_CORES):
        in2.append({
            "y_t": np.ascontiguousarray(y_all[:, i * RPC:(i + 1) * RPC]),
            "x_rows": np.ascontiguousarray(xf[i * RPC:(i + 1) * RPC]),
            **in2_common,
        })
    res2 = run_bass_kernel_spmd(nc2, in2, core_ids=list(range(N_CORES)),
                                **kwargs)
    outf = np.concatenate(
        [np.asarray(res2.results[i]["out"]) for i in range(N_CORES)], axis=0)
    return outf.reshape(B, T, C).astype(np.float32), res1, res2


def kernel(**inputs):
    out, _, _ = run(inputs, trace=False)
    return out


# revision 17
# speedup vs baseline: 1.1500x; 1.1500x over previous
"""Trainium2 Bass kernel for nn_Block_24111946399747 (dense transformer block).

Strategy (8 NeuronCores, two SPMD launches; heads->rows reshard on host):

Launch 1 (head-sharded attention; core i owns heads 2i, 2i+1):
  - Pipelined per 512-token block: LN stats (DVE bn_stats on f32 x) ->
    LN apply (ACT, f32 -> fp8 e4m3 h) -> blocked DRAM store -> fast
    contiguous DMA transposes (split across sync+scalar HWDGE queues) ->
    fp8 DoubleRow QKV matmuls (256-contraction per pass, weights x64).
  - Attention (bf16): S^T = k^T q per 128-key block, exp fused over
    pairs of blocks on ACT (PSUM 2-bank span), causal mask on DVE,
    y^T = v^T P^T and rowsum via ones-matmul accumulated in PSUM.
    Epilogue per (b, head): batched DVE reciprocal of rowsums, K=1
    ones-matmul broadcast of 1/rowsum across partitions, DVE scale,
    y_t emitted as fp8.

Host: stack per-core y_t -> y_all [2048, 4096] fp8, hand each core its
512-token column slice.

Launch 2 (row-sharded; core i owns token rows 512i..512i+512):
  - proj in fp8 DoubleRow (y pairs stacked via AP rearrange, w_pr x64):
    x2 = y^T w_pr/64 + (x + b_pr), residual kept f32 in SBUF.
  - x2 -> bf16 blocked DRAM -> fast transposes -> x2_t feature-major.
  - MLP in bf16: m = Gelu(w_fc^T x2_t + b_fc) (ACT), out = m^T w_out
    + (x2 + b_out), token-major f32 out; host concatenates.
"""

import math
import os
import sys

import numpy as np

if "/opt/trn_rl_repo" not in sys.path:
    sys.path.insert(0, "/opt/trn_rl_repo")

import ml_dtypes  # noqa: E402

import concourse.bass as bass  # noqa: E402,F401
import concourse.mybir as mybir  # noqa: E402
import concourse.tile as tile  # noqa: E402
from concourse import bacc  # noqa: E402
from concourse.bass_utils import run_bass_kernel_spmd  # noqa: E402

B, T, C, H = 2, 2048, 2048, 16
HD = C // H            # 128 head dim
N_CORES = 8
HPC = H // N_CORES     # 2 heads per core
NTOK = B * T           # 4096 tokens
RPC = NTOK // N_CORES  # 512 rows per core
P = 128
KC = C // P            # 16 contraction chunks over C
KP = C // 256          # 8 fp8-pair chunks over C
F4 = 4 * C             # 8192
MC4 = F4 // P          # 64 contraction chunks over 4C
NT = NTOK // P         # 32 token tiles of 128
NTT = NTOK // 512      # 8 token tiles of 512
JTT = T // 512         # 4 query tiles of 512 per batch
EPS = 1e-6
WS = 64.0              # fp8 weight scale
BF16 = mybir.dt.bfloat16
F8 = mybir.dt.float8e4
F32 = mybir.dt.float32
ALU = mybir.AluOpType
ACT = mybir.ActivationFunctionType
DR = mybir.MatmulPerfMode.DoubleRow

_BUILT1 = None
_BUILT2 = None
# Phase gating for bisection: prefix of "AC" (launch 1) / "DE" (launch 2).
_PHASES = os.environ.get("KERNEL_PHASES", "ACDE")


# ======================= Launch 1: LN + QKV + attention =======================

def _emit_attn(nc, tc, io):
    x_full = io["x_full"]
    w_qk, b_qk, w_v, b_v = io["w_qk"], io["b_qk"], io["w_v"], io["b_v"]
    cmask, yt_out = io["cmask"], io["yt"]
    ph = _PHASES

    from contextlib import ExitStack

    with ExitStack() as es:
        constp = es.enter_context(tc.tile_pool(name="constp", bufs=1))
        dramp = es.enter_context(tc.tile_pool(name="dramp", bufs=1,
                                              space="DRAM"))
        ones_sb = constp.tile([P, P], BF16, name="ones_sb")
        nc.any.memset(ones_sb[:], 1.0)
        eps_sb = constp.tile([P, 1], F32, name="eps_sb")
        nc.any.memset(eps_sb[:], EPS)
        mask_sb = constp.tile([P, P], BF16, name="mask_sb")
        nc.sync.dma_start(mask_sb[:], cmask[:, :])
        b_qk_sb = constp.tile([P, 4], F32, name="b_qk_sb")
        nc.sync.dma_start(b_qk_sb[:], b_qk.ap().rearrange("(c p) -> p c", p=P))
        b_v_sb = constp.tile([P, HPC], F32, name="b_v_sb")
        nc.sync.dma_start(b_v_sb[:], b_v.ap().rearrange("(c p) -> p c", p=P))

        # persistent QKV outputs (allocated before transient pools)
        persbc = es.enter_context(tc.tile_pool(name="persbc", bufs=1))
        qk_t = persbc.tile([P, 4, NTOK], BF16, name="qk_t")
        v_sb = persbc.tile([P, NT, HPC * HD], BF16, name="v_sb")
        wqp = es.enter_context(tc.tile_pool(name="wqp", bufs=1))
        w_qk_sb = wqp.tile([P, KP, 2, 4 * P], F8, name="w_qk_sb")
        nc.sync.dma_start(
            w_qk_sb[:], w_qk.ap().rearrange("ko p two f -> p ko two f"))
        w_v_sb = wqp.tile([P, KP, 2, HPC * HD], F8, name="w_v_sb")
        nc.sync.dma_start(
            w_v_sb[:], w_v.ap().rearrange("ko p two f -> p ko two f"))

        # blocked h storage: per 1024-token block pair, KP chunks of
        # [1024, 128] u16 pairs (contiguous transpose sources)
        h_blks = [dramp.tile([KP, 1024, P], BF16, name=f"h_blk{gp}")
                  for gp in range(NTT // 2)]
        # blocked v storage: per pair, [t8] chunks of [256 feat, 128 tok]
        v_blks = [dramp.tile([8, HPC * P, P], BF16, name=f"v_blk{gp}")
                  for gp in range(NTT // 2)]

        # ---------------- Phase A: LN + QKV, pipelined per block pair -----
        GRP = 4
        inv_ws = 1.0 / WS
        with tc.tile_pool(name="lnp", bufs=GRP + 2) as lnp, \
             tc.tile_pool(name="lnw", bufs=3) as lnw, \
             tc.tile_pool(name="lns", bufs=2) as lns, \
             tc.tile_pool(name="hfp", bufs=3) as hfp, \
             tc.tile_pool(name="htp", bufs=2) as htp, \
             tc.tile_pool(name="vtp", bufs=2) as vtp, \
             tc.tile_pool(name="qps", bufs=2, space="PSUM") as qps, \
             tc.tile_pool(name="vps", bufs=2, space="PSUM") as vps:
            for gp in range(NTT // 2):
                for g2 in range(2):  # the two 512-token blocks of this pair
                    g = 2 * gp + g2
                    # --- LN for the 4 token tiles of this block ---
                    xts = []
                    mvg = lns.tile([P, GRP, 2], F32, tag="mvg")
                    for j in range(GRP):
                        t = g * GRP + j
                        xt = lnp.tile([P, C], F32, tag="xt")
                        nc.gpsimd.dma_start(xt[:],
                                            x_full[t * P:(t + 1) * P, :])
                        xts.append(xt)
                        stats = lnw.tile([P, 4, 6], F32, tag="stats")
                        xr = xt[:].rearrange("p (s f) -> p s f", f=512)
                        for s in range(4):
                            nc.vector.bn_stats(stats[:, s, :], xr[:, s, :])
                        nc.vector.bn_aggr(mvg[:, j, :], stats[:])
                    stdg = lns.tile([P, GRP], F32, tag="stdg")
                    nc.scalar.activation(stdg[:], mvg[:, :, 1], ACT.Sqrt,
                                         bias=eps_sb[:])
                    rstdg = lns.tile([P, GRP], F32, tag="rstdg")
                    nc.vector.reciprocal(rstdg[:], stdg[:])
                    nmrg = lns.tile([P, GRP], F32, tag="nmrg")
                    nc.vector.tensor_mul(nmrg[:], mvg[:, :, 0], rstdg[:])
                    nc.vector.tensor_scalar_mul(nmrg[:], nmrg[:], -1.0)
                    for j in range(GRP):
                        ht = hfp.tile([P, C], F8, tag="ht")
                        nc.scalar.activation(ht[:], xts[j][:], ACT.Identity,
                                             bias=nmrg[:, j:j + 1],
                                             scale=rstdg[:, j:j + 1])
                        # store as u16 pairs into blocked DRAM
                        nc.gpsimd.dma_start(
                            h_blks[gp][:, (g2 * GRP + j) * P:
                                       (g2 * GRP + j + 1) * P, :]
                            .rearrange("c t f -> t c f"),
                            ht[:].bitcast(BF16)
                            .rearrange("p (c f) -> p c f", f=P))
                # --- batched transposes, sync queue only ---
                h_t = htp.tile([P, KP, 1024], BF16, tag="h_t")
                for ko in range(KP):
                    nc.sync.dma_start_transpose(h_t[:, ko, :],
                                                h_blks[gp][ko, :, :])
                # fp8 pair view: [p][ko][two][tok] over 1024 tokens
                hp = (h_t[:].bitcast(F8)
                      .rearrange("p c (t two) -> p c two t", two=2))
                # --- QKV DoubleRow matmuls (two 512-token halves) ---
                v_td = vtp.tile([P, HPC, 1024], BF16, tag="v_td")
                for g2 in range(2):
                    g = 2 * gp + g2
                    hsl = hp[:, :, :, g2 * 512:(g2 + 1) * 512]
                    for fc in range(4):  # q0,q1,k0,k1 feature chunks
                        ps = qps.tile([P, 512], F32, tag="qk_ps")
                        for ko in range(KP):
                            nc.tensor.matmul(
                                ps[:], w_qk_sb[:, ko, :, fc * P:(fc + 1) * P],
                                hsl[:, ko],
                                start=(ko == 0), stop=(ko == KP - 1),
                                perf_mode=DR)
                        nc.scalar.activation(
                            qk_t[:, fc, g * 512:(g + 1) * 512], ps[:],
                            ACT.Identity, bias=b_qk_sb[:, fc:fc + 1],
                            scale=inv_ws)
                    for fc in range(HPC):  # v feature-major via DR
                        psv = vps.tile([P, 512], F32, tag="v_ps")
                        for ko in range(KP):
                            nc.tensor.matmul(
                                psv[:], w_v_sb[:, ko, :, fc * P:(fc + 1) * P],
                                hsl[:, ko],
                                start=(ko == 0), stop=(ko == KP - 1),
                                perf_mode=DR)
                        nc.scalar.activation(
                            v_td[:, fc, g2 * 512:(g2 + 1) * 512], psv[:],
                            ACT.Identity, bias=b_v_sb[:, fc:fc + 1],
                            scale=inv_ws)
                # v blocked store: [t8][fc*128+p][tok]
                for fc in range(HPC):
                    nc.gpsimd.dma_start(
                        v_blks[gp][:, fc * P:(fc + 1) * P, :]
                        .rearrange("t p k -> p t k"),
                        v_td[:, fc, :].rearrange("p (t k) -> p t k", k=P))
                for t8 in range(8):
                    nc.sync.dma_start_transpose(
                        v_sb[:, gp * 8 + t8, :], v_blks[gp][t8])

        if "C" not in ph:  # dump q_t head 0 into yt and stop
            with tc.tile_pool(name="dmp", bufs=2) as dmp:
                for rb in range(2):
                    t = dmp.tile([P, NTOK], F8, tag="t")
                    nc.vector.tensor_copy(t[:], qk_t[:, rb, :])
                    nc.sync.dma_start(yt_out[rb * P:(rb + 1) * P, :], t[:])
            return

        # ---------------- Phase C: causal attention ----------------
        with tc.tile_pool(name="sps", bufs=2, space="PSUM") as sps, \
             tc.tile_pool(name="yps", bufs=2, space="PSUM") as yps, \
             tc.tile_pool(name="rps", bufs=2, space="PSUM") as rps, \
             tc.tile_pool(name="attp", bufs=2) as attp, \
             tc.tile_pool(name="rvp", bufs=2) as rvp, \
             tc.tile_pool(name="yfp", bufs=3) as yfp:
            inv_sqrt_hd = 1.0 / math.sqrt(HD)
            for b in range(B):
                for hl in range(HPC):
                    q_sl = qk_t[:, hl, b * T:(b + 1) * T]
                    k_sl = qk_t[:, 2 + hl, b * T:(b + 1) * T]
                    for jt in range(JTT):
                        nblk = 4 * (jt + 1)
                        pt = attp.tile([P, 16, 512], BF16, tag="pt")
                        y_ps = yps.tile([P, 512], F32, tag="y_ps")
                        # all-ones stationary: every partition of rs_ps
                        # receives the rowsum (broadcast inside the matmul)
                        rs_ps = rps.tile([P, 512], F32, tag="rs_ps")
                        for ib2 in range(nblk // 2):
                            sp = sps.tile([P, 1024], F32, tag="s_ps")
                            c0s = []
                            for u in range(2):
                                ib = 2 * ib2 + u
                                c0 = max(0, ib * P - jt * 512)
                                c0s.append(c0)
                                nc.tensor.matmul(
                                    sp[:, u * 512 + c0:(u + 1) * 512],
                                    k_sl[:, ib * P:(ib + 1) * P],
                                    q_sl[:, jt * 512 + c0:(jt + 1) * 512],
                                    start=True, stop=True)
                            c0a = c0s[0]
                            pt2 = pt[:, 2 * ib2:2 * ib2 + 2, :].rearrange(
                                "p a b -> p (a b)")
                            nc.scalar.activation(
                                pt2[:, c0a:1024], sp[:, c0a:1024],
                                ACT.Exp, scale=inv_sqrt_hd)
                            for u in range(2):
                                ib = 2 * ib2 + u
                                c0 = c0s[u]
                                if ib >= 4 * jt:  # diagonal 128x128 sub-block
                                    nc.vector.tensor_mul(
                                        pt[:, ib, c0:c0 + P],
                                        pt[:, ib, c0:c0 + P], mask_sb[:])
                                vv = v_sb[:, b * (T // P) + ib,
                                          hl * HD:(hl + 1) * HD]
                                nc.tensor.matmul(
                                    y_ps[:, c0:512], vv, pt[:, ib, c0:512],
                                    start=(ib == 0), stop=(ib == nblk - 1))
                                nc.tensor.matmul(
                                    rs_ps[:, c0:512], ones_sb[:],
                                    pt[:, ib, c0:512],
                                    start=(ib == 0), stop=(ib == nblk - 1))
                        rinv = rvp.tile([P, 512], F32, tag="rinv")
                        nc.vector.reciprocal_approx_fast(rinv[:], rs_ps[:])
                        yf = yfp.tile([P, 512], F8, tag="yf")
                        nc.vector.tensor_mul(yf[:], y_ps[:], rinv[:])
                        nc.gpsimd.dma_start(
                            yt_out[hl * HD:(hl + 1) * HD,
                                   b * T + jt * 512:b * T + (jt + 1) * 512],
                            yf[:])


def _build_attn():
    nc = bacc.Bacc("TRN2", target_bir_lowering=False, debug=False,
                   num_devices=N_CORES)
    io = {}
    io["x_full"] = nc.dram_tensor("x_full", [NTOK, C], F32,
                                  kind="ExternalInput").ap()
    io["w_qk"] = nc.dram_tensor("w_qk", [KP, P, 2, 4 * P], F8,
                                kind="ExternalInput")
    io["b_qk"] = nc.dram_tensor("b_qk", [4 * P], F32, kind="ExternalInput")
    io["w_v"] = nc.dram_tensor("w_v", [KP, P, 2, HPC * HD], F8,
                               kind="ExternalInput")
    io["b_v"] = nc.dram_tensor("b_v", [HPC * HD], F32, kind="ExternalInput")
    io["cmask"] = nc.dram_tensor("cmask", [P, P], BF16,
                                 kind="ExternalInput").ap()
    io["yt"] = nc.dram_tensor("yt", [HPC * HD, NTOK], F8,
                              kind="ExternalOutput").ap()
    with tile.TileContext(nc) as tc:
        _emit_attn(nc, tc, io)
    nc.compile()
    return nc


# ======================= Launch 2: proj + MLP =======================

def _emit_mlp(nc, tc, io):
    y_t_in, x_rows = io["y_t"], io["x_rows"]
    w_pr, b_pr = io["w_pr"], io["b_pr"]
    w_fc, b_fc, w_out, b_out = io["w_fc"], io["b_fc"], io["w_out"], io["b_out"]
    out = io["out"]
    ph = _PHASES
    inv_ws = 1.0 / WS

    from contextlib import ExitStack

    with ExitStack() as es:
        constp = es.enter_context(tc.tile_pool(name="constp", bufs=1))
        dramp = es.enter_context(tc.tile_pool(name="dramp", bufs=1,
                                              space="DRAM"))
        b_fc_sb = constp.tile([P, MC4], F32, name="b_fc_sb")
        nc.sync.dma_start(b_fc_sb[:], b_fc.ap().rearrange("(c p) -> p c", p=P))
        b_pr_sb = constp.tile([P, C], F32, name="b_pr_sb")
        nc.scalar.dma_start(b_pr_sb[:], b_pr.ap()[None, :].to_broadcast((P, C)))
        b_out_sb = constp.tile([P, C], F32, name="b_out_sb")
        nc.scalar.dma_start(b_out_sb[:], b_out.ap()[None, :].to_broadcast((P, C)))

        # blocked x2 storage: KC chunks of [512, 128] bf16
        x2_blk = dramp.tile([KC, 512, P], BF16, name="x2_blk")

        persde = es.enter_context(tc.tile_pool(name="persde", bufs=1))
        x2_f32 = persde.tile([P, 4, C], F32, name="x2_f32")
        x2_t = persde.tile([P, KC, RPC], BF16, name="x2_t")
        y_sb = persde.tile([P, KP, 2, RPC], F8, name="y_sb")

        # ---------------- Phase D: proj + residual ----------------
        with tc.tile_pool(name="pdp", bufs=3) as pdp, \
             tc.tile_pool(name="wprp", bufs=2) as wprp, \
             tc.tile_pool(name="dps", bufs=3, space="PSUM") as dps:
            y_re = y_t_in.ap().rearrange("(ko i p) r -> p ko i r", p=P, i=2)
            for ko in range(KP):
                nc.sync.dma_start(y_sb[:, ko], y_re[:, ko])
            for rb in range(4):
                nc.scalar.dma_start(
                    x2_f32[:, rb, :], x_rows[rb * P:(rb + 1) * P, :])
            for rb in range(4):
                nc.vector.tensor_add(
                    x2_f32[:, rb, :], x2_f32[:, rb, :], b_pr_sb[:])
            for ct in range(4):
                wt = wprp.tile([P, KP, 2, 512], F8, tag="wpr")
                nc.sync.dma_start(
                    wt[:], w_pr.ap()[ct].rearrange("ko p two f -> p ko two f"))
                for rb in range(4):
                    ps = dps.tile([P, 512], F32, tag="pr_ps")
                    for ko in range(KP):
                        nc.tensor.matmul(
                            ps[:], y_sb[:, ko, :, rb * P:(rb + 1) * P],
                            wt[:, ko],
                            start=(ko == 0), stop=(ko == KP - 1),
                            perf_mode=DR)
                    sl = x2_f32[:, rb, ct * 512:(ct + 1) * 512]
                    nc.vector.scalar_tensor_tensor(
                        sl, ps[:], inv_ws, sl, op0=ALU.mult, op1=ALU.add)
                    x2b = pdp.tile([P, 512], BF16, tag="x2b")
                    nc.vector.tensor_copy(x2b[:], sl)
                    nc.gpsimd.dma_start(
                        x2_blk[ct * 4:(ct + 1) * 4, rb * P:(rb + 1) * P, :]
                        .rearrange("c t f -> t c f"),
                        x2b[:].rearrange("p (c f) -> p c f", f=P))
                # transpose-load this ct's feature chunks (pairs: [1024,128])
                for kk in range(2):
                    nc.sync.dma_start_transpose(
                        x2_t[:, 4 * ct + 2 * kk:4 * ct + 2 * kk + 2, :]
                        .rearrange("p c t -> p (c t)"),
                        x2_blk[4 * ct + 2 * kk:4 * ct + 2 * kk + 2, :, :]
                        .rearrange("c t f -> (c t) f"))

        if "E" not in ph:  # dump x2 and stop
            with tc.tile_pool(name="dmp", bufs=2) as dmp:
                for rb in range(4):
                    nc.sync.dma_start(
                        out[rb * P:(rb + 1) * P, :], x2_f32[:, rb, :])
            return

        # pre-add b_out into the residual (after proj phase)
        for rb in range(4):
            nc.vector.tensor_add(
                x2_f32[:, rb, :], x2_f32[:, rb, :], b_out_sb[:])

        # ---------------- Phase E: MLP + residual ----------------
        with tc.tile_pool(name="mep", bufs=1) as mep, \
             tc.tile_pool(name="wfcp", bufs=4) as wfcp, \
             tc.tile_pool(name="wop", bufs=8) as wop, \
             tc.tile_pool(name="ofp", bufs=3) as ofp, \
             tc.tile_pool(name="eps1", bufs=3, space="PSUM") as eps1, \
             tc.tile_pool(name="eps2", bufs=1, space="PSUM") as eps2:
            m_sb = mep.tile([P, MC4, RPC], BF16, name="m_sb")
            for mc in range(MC4):
                wfc_t = wfcp.tile([P, KC, P], BF16, tag="wfc")
                nc.sync.dma_start(
                    wfc_t[:],
                    w_fc.ap()[:, mc * P:(mc + 1) * P]
                    .rearrange("(ko p) m -> p ko m", p=P))
                ps = eps1.tile([P, 512], F32, tag="fc_ps")
                for ko in range(KC):
                    nc.tensor.matmul(
                        ps[:], wfc_t[:, ko, :], x2_t[:, ko, :],
                        start=(ko == 0), stop=(ko == KC - 1))
                nc.scalar.activation(
                    m_sb[:, mc, :], ps[:], ACT.Gelu,
                    bias=b_fc_sb[:, mc:mc + 1], scale=1.0)
            if "F" in ph:  # debug: dump m chunks 0..15 feature-major
                with tc.tile_pool(name="dmp2", bufs=2) as dmp2:
                    for mc in range(16):
                        mf = dmp2.tile([P, RPC], F32, tag="mf")
                        nc.vector.tensor_copy(mf[:], m_sb[:, mc, :])
                        nc.sync.dma_start(
                            out[(mc % 4) * P:(mc % 4 + 1) * P,
                                (mc // 4) * 512:(mc // 4 + 1) * 512],
                            mf[:])
                return
            for ct in range(4):
                pss = [eps2.tile([P, 512], F32, tag=f"o_ps{rb}",
                                 name=f"o_ps{rb}_{ct}")
                       for rb in range(4)]
                for ko in range(MC4):
                    wo_t = wop.tile([P, 512], BF16, tag="wo")
                    nc.sync.dma_start(
                        wo_t[:],
                        w_out.ap()[ko * P:(ko + 1) * P,
                                   ct * 512:(ct + 1) * 512])
                    for rb in range(4):
                        nc.tensor.matmul(
                            pss[rb][:], m_sb[:, ko, rb * P:(rb + 1) * P],
                            wo_t[:],
                            start=(ko == 0), stop=(ko == MC4 - 1))
                for rb in range(4):
                    of = ofp.tile([P, 512], F32, tag="of")
                    nc.vector.tensor_add(
                        of[:], pss[rb][:],
                        x2_f32[:, rb, ct * 512:(ct + 1) * 512])
                    nc.scalar.dma_start(
                        out[rb * P:(rb + 1) * P, ct * 512:(ct + 1) * 512],
                        of[:])


def _build_mlp():
    nc = bacc.Bacc("TRN2", target_bir_lowering=False, debug=False,
                   num_devices=N_CORES)
    io = {}
    io["y_t"] = nc.dram_tensor("y_t", [C, RPC], F8, kind="ExternalInput")
    io["x_rows"] = nc.dram_tensor("x_rows", [RPC, C], F32,
                                  kind="ExternalInput").ap()
    io["w_pr"] = nc.dram_tensor("w_pr", [4, KP, P, 2, 512], F8,
                                kind="ExternalInput")
    io["b_pr"] = nc.dram_tensor("b_pr", [C], F32, kind="ExternalInput")
    io["w_fc"] = nc.dram_tensor("w_fc", [C, F4], BF16, kind="ExternalInput")
    io["b_fc"] = nc.dram_tensor("b_fc", [F4], F32, kind="ExternalInput")
    io["w_out"] = nc.dram_tensor("w_out", [F4, C], BF16, kind="ExternalInput")
    io["b_out"] = nc.dram_tensor("b_out", [C], F32, kind="ExternalInput")
    io["out"] = nc.dram_tensor("out", [RPC, C], F32,
                               kind="ExternalOutput").ap()
    with tile.TileContext(nc) as tc:
        _emit_mlp(nc, tc, io)
    nc.compile()
    return nc


def _get_built():
    global _BUILT1, _BUILT2
    if _BUILT1 is None:
        _BUILT1 = _build_attn()
    if _BUILT2 is None and any(p in _PHASES for p in "DE"):
        _BUILT2 = _build_mlp()
    return _BUILT1, _BUILT2


# ======================= Host orchestration =======================

def _pack_pairs_interleaved(w, scale):
    """[C, F] -> [KP, 128, 2, F] fp8, pair rows (256k+2j, 256k+2j+1)."""
    f8 = ml_dtypes.float8_e4m3
    wf = np.asarray(w, np.float64) * scale
    Cd, F = wf.shape
    wr = wf.reshape(Cd // 256, 128, 2, F)  # [ko][j][i] = row 256ko+2j+i
    return np.ascontiguousarray(wr.astype(np.float32)).astype(f8)


def _pack_pairs_stacked(w, scale):
    """[C, F] -> [KP, 128, 2, F] fp8, pair rows (256k+j, 256k+128+j)."""
    f8 = ml_dtypes.float8_e4m3
    wf = np.asarray(w, np.float64) * scale
    Cd, F = wf.shape
    wr = wf.reshape(Cd // 256, 2, 128, F).transpose(0, 2, 1, 3)
    return np.ascontiguousarray(wr.astype(np.float32)).astype(f8)


def _prep(x, ln_scale, ln_bias, w_qkv, b_qkv, w_proj, b_proj,
          w_fc, b_fc, w_out, b_out):
    bf = ml_dtypes.bfloat16
    xf = np.ascontiguousarray(np.asarray(x, np.float32).reshape(NTOK, C))
    # Fold LN affine into the QKV projection (exact, in float64).
    w64 = np.asarray(w_qkv, np.float64)
    g = np.asarray(ln_scale, np.float64)
    beta = np.asarray(ln_bias, np.float64)
    w_eff = g[:, None] * w64
    b_eff = np.asarray(b_qkv, np.float64) + beta @ w64

    wq, wk, wv = w_eff[:, :C], w_eff[:, C:2 * C], w_eff[:, 2 * C:]
    bq, bk, bv = b_eff[:C], b_eff[C:2 * C], b_eff[2 * C:]
    cmask = np.triu(np.ones((P, P), np.float32)).astype(bf)

    in1 = []
    for i in range(N_CORES):
        hs = slice(i * HPC * HD, (i + 1) * HPC * HD)
        w_qk_i = _pack_pairs_interleaved(
            np.concatenate([wq[:, hs], wk[:, hs]], axis=1), WS)
        b_qk_i = np.ascontiguousarray(
            np.concatenate([bq[hs], bk[hs]]).astype(np.float32))
        w_v_i = _pack_pairs_interleaved(wv[:, hs], WS)
        b_v_i = np.ascontiguousarray(bv[hs].astype(np.float32))
        in1.append({
            "x_full": xf,
            "w_qk": w_qk_i, "b_qk": b_qk_i, "w_v": w_v_i, "b_v": b_v_i,
            "cmask": cmask,
        })

    # proj weights: [4ct][KP, 128, 2, 512] fp8, stacked pairing
    wp = _pack_pairs_stacked(np.asarray(w_proj, np.float64), WS)  # [8,128,2,2048]
    w_pr_b = np.ascontiguousarray(
        wp.reshape(KP, P, 2, 4, 512).transpose(3, 0, 1, 2, 4))
    w_fc_b = np.asarray(w_fc, np.float32).astype(bf)
    w_out_b = np.asarray(w_out, np.float32).astype(bf)
    b_pr_f = np.ascontiguousarray(np.asarray(b_proj, np.float32))
    b_fc_f = np.ascontiguousarray(np.asarray(b_fc, np.float32))
    b_out_f = np.ascontiguousarray(np.asarray(b_out, np.float32))
    in2_common = {
        "w_pr": w_pr_b, "b_pr": b_pr_f, "w_fc": w_fc_b, "b_fc": b_fc_f,
        "w_out": w_out_b, "b_out": b_out_f,
    }
    return xf, in1, in2_common


def run(inputs, trace=False, trace_cores=None):
    """Run both SPMD launches. Returns (output [B,T,C] f32, res1, res2)."""
    nc1, nc2 = _get_built()
    xf, in1, in2_common = _prep(**inputs)
    kwargs = {}
    if trace:
        kwargs = dict(trace=True,
                      trace_cores=trace_cores if trace_cores else [0])
    res1 = run_bass_kernel_spmd(nc1, in1, core_ids=list(range(N_CORES)),
                                **kwargs)
    y_all = np.concatenate(
        [np.asarray(res1.results[i]["yt"]) for i in range(N_CORES)], axis=0)
    if nc2 is None:
        return y_all, res1, None

    in2 = []
    for i in range(N_CORES):
        in2.append({
            "y_t": np.ascontiguousarray(y_all[:, i * RPC:(i + 1) * RPC]),
            "x_rows": np.ascontiguousarray(xf[i * RPC:(i + 1) * RPC]),
            **in2_common,
        })
    res2 = run_bass_kernel_spmd(nc2, in2, core_ids=list(range(N_CORES)),
                                **kwargs)
    outf = np.concatenate(
        [np.asarray(res2.results[i]["out"]) for i in range(N_CORES)], axis=0)
    return outf.reshape(B, T, C).astype(np.float32), res1, res2


def kernel(**inputs):
    out, _, _ = run(inputs, trace=False)
    return out


# revision 20
# speedup vs baseline: 1.1843x; 1.0298x over previous
"""Trainium2 Bass kernel for nn_Block_24111946399747 (dense transformer block).

Strategy (8 NeuronCores, two SPMD launches; heads->rows reshard on host):

Launch 1 (head-sharded attention; core i owns heads 2i, 2i+1):
  - Pipelined per 512-token block: LN stats (DVE bn_stats on f32 x) ->
    LN apply (ACT, f32 -> fp8 e4m3 h) -> blocked DRAM store -> fast
    contiguous DMA transposes (split across sync+scalar HWDGE queues) ->
    fp8 DoubleRow QKV matmuls (256-contraction per pass, weights x64).
  - Attention (bf16): S^T = k^T q per 128-key block, exp fused over
    pairs of blocks on ACT (PSUM 2-bank span), causal mask on DVE,
    y^T = v^T P^T and rowsum via ones-matmul accumulated in PSUM.
    Epilogue per (b, head): batched DVE reciprocal of rowsums, K=1
    ones-matmul broadcast of 1/rowsum across partitions, DVE scale,
    y_t emitted as fp8.

Host: stack per-core y_t -> y_all [2048, 4096] fp8, hand each core its
512-token column slice.

Launch 2 (row-sharded; core i owns token rows 512i..512i+512):
  - proj in fp8 DoubleRow (y pairs stacked via AP rearrange, w_pr x64):
    x2 = y^T w_pr/64 + (x + b_pr), residual kept f32 in SBUF.
  - x2 -> bf16 blocked DRAM -> fast transposes -> x2_t feature-major.
  - MLP in bf16: m = Gelu(w_fc^T x2_t + b_fc) (ACT), out = m^T w_out
    + (x2 + b_out), token-major f32 out; host concatenates.
"""

import math
import os
import sys

import numpy as np

if "/opt/trn_rl_repo" not in sys.path:
    sys.path.insert(0, "/opt/trn_rl_repo")

import ml_dtypes  # noqa: E402

import concourse.bass as bass  # noqa: E402,F401
import concourse.mybir as mybir  # noqa: E402
import concourse.tile as tile  # noqa: E402
from concourse import bacc  # noqa: E402
from concourse.bass_utils import run_bass_kernel_spmd  # noqa: E402

B, T, C, H = 2, 2048, 2048, 16
HD = C // H            # 128 head dim
N_CORES = 8
HPC = H // N_CORES     # 2 heads per core
NTOK = B * T           # 4096 tokens
RPC = NTOK // N_CORES  # 512 rows per core
P = 128
KC = C // P            # 16 contraction chunks over C
KP = C // 256          # 8 fp8-pair chunks over C
F4 = 4 * C             # 8192
MC4 = F4 // P          # 64 contraction chunks over 4C
NT = NTOK // P         # 32 token tiles of 128
NTT = NTOK // 512      # 8 token tiles of 512
JTT = T // 512         # 4 query tiles of 512 per batch
EPS = 1e-6
WS = 64.0              # fp8 weight scale
BF16 = mybir.dt.bfloat16
F8 = mybir.dt.float8e4
F32 = mybir.dt.float32
ALU = mybir.AluOpType
ACT = mybir.ActivationFunctionType
DR = mybir.MatmulPerfMode.DoubleRow

_BUILT1 = None
_BUILT2 = None
# Phase gating for bisection: prefix of "AC" (launch 1) / "DE" (launch 2).
_PHASES = os.environ.get("KERNEL_PHASES", "ACDE")


# ======================= Launch 1: LN + QKV + attention =======================

def _emit_attn(nc, tc, io):
    x_full = io["x_full"]
    w_qk, b_qk, w_v, b_v = io["w_qk"], io["b_qk"], io["w_v"], io["b_v"]
    cmask, yt_out = io["cmask"], io["yt"]
    ph = _PHASES

    from contextlib import ExitStack

    with ExitStack() as es:
        constp = es.enter_context(tc.tile_pool(name="constp", bufs=1))
        dramp = es.enter_context(tc.tile_pool(name="dramp", bufs=1,
                                              space="DRAM"))
        ones_sb = constp.tile([P, P], BF16, name="ones_sb")
        nc.any.memset(ones_sb[:], 1.0)
        eps_sb = constp.tile([P, 1], F32, name="eps_sb")
        nc.any.memset(eps_sb[:], EPS)
        mask_sb = constp.tile([P, P], BF16, name="mask_sb")
        nc.sync.dma_start(mask_sb[:], cmask[:, :])
        b_qk_sb = constp.tile([P, 4], F32, name="b_qk_sb")
        nc.sync.dma_start(b_qk_sb[:], b_qk.ap().rearrange("(c p) -> p c", p=P))
        b_v_sb = constp.tile([P, HPC], F32, name="b_v_sb")
        nc.sync.dma_start(b_v_sb[:], b_v.ap().rearrange("(c p) -> p c", p=P))

        # persistent QKV outputs (allocated before transient pools)
        persbc = es.enter_context(tc.tile_pool(name="persbc", bufs=1))
        qk_t = persbc.tile([P, 4, NTOK], BF16, name="qk_t")
        v_sb = persbc.tile([P, NT, HPC * HD], BF16, name="v_sb")
        wqp = es.enter_context(tc.tile_pool(name="wqp", bufs=1))
        w_qk_sb = wqp.tile([P, KP, 2, 4 * P], F8, name="w_qk_sb")
        nc.sync.dma_start(
            w_qk_sb[:], w_qk.ap().rearrange("ko p two f -> p ko two f"))
        w_v_sb = wqp.tile([P, KP, 2, HPC * HD], F8, name="w_v_sb")
        nc.sync.dma_start(
            w_v_sb[:], w_v.ap().rearrange("ko p two f -> p ko two f"))

        # blocked h storage: per 1024-token block pair, KP chunks of
        # [1024, 128] u16 pairs (contiguous transpose sources)
        h_blks = [dramp.tile([KP, 1024, P], BF16, name=f"h_blk{gp}")
                  for gp in range(NTT // 2)]
        # blocked v storage: per pair, [t8] chunks of [256 feat, 128 tok]
        v_blks = [dramp.tile([8, HPC * P, P], BF16, name=f"v_blk{gp}")
                  for gp in range(NTT // 2)]

        # ---------------- Phase A: LN + QKV, pipelined per block pair -----
        # Software-pipelined over 1024-token block pairs. All x loads are
        # SWDGE cast-DMAs (f32 -> bf16); per-pair emit order keeps the
        # gpsimd FIFO free of matmul-dependent head-of-line blocking:
        #   LN(gp) -> h stores(gp) -> h transposes(gp) -> v transposes(gp-1)
        #   -> x loads(gp+1) -> QKV matmuls(gp) -> v stores(gp)
        GRP = 4
        NPAIR = NTT // 2
        inv_ws = 1.0 / WS

        def load_pair(lnp, gp):
            xts = []
            for tt in range(2 * GRP):
                t = gp * 2 * GRP + tt
                xt = lnp.tile([P, C], BF16, tag="xt")
                nc.gpsimd.dma_start(xt[:], x_full[t * P:(t + 1) * P, :])
                xts.append(xt)
            return xts

        with tc.tile_pool(name="lnp", bufs=2 * GRP + 6) as lnp, \
             tc.tile_pool(name="lnw", bufs=3) as lnw, \
             tc.tile_pool(name="lns", bufs=2) as lns, \
             tc.tile_pool(name="hfp", bufs=3) as hfp, \
             tc.tile_pool(name="htp", bufs=2) as htp, \
             tc.tile_pool(name="vtp", bufs=2) as vtp, \
             tc.tile_pool(name="qps", bufs=2, space="PSUM") as qps, \
             tc.tile_pool(name="vps", bufs=2, space="PSUM") as vps:
            xts = load_pair(lnp, 0)
            for gp in range(NPAIR):
                # --- LN for the 8 token tiles of this pair ---
                for g2 in range(2):
                    g = 2 * gp + g2
                    mvg = lns.tile([P, GRP, 2], F32, tag="mvg")
                    for j in range(GRP):
                        xt = xts[g2 * GRP + j]
                        stats = lnw.tile([P, 4, 6], F32, tag="stats")
                        xr = xt[:].rearrange("p (s f) -> p s f", f=512)
                        for s in range(4):
                            nc.vector.bn_stats(stats[:, s, :], xr[:, s, :])
                        nc.vector.bn_aggr(mvg[:, j, :], stats[:])
                    stdg = lns.tile([P, GRP], F32, tag="stdg")
                    nc.scalar.activation(stdg[:], mvg[:, :, 1], ACT.Sqrt,
                                         bias=eps_sb[:])
                    rstdg = lns.tile([P, GRP], F32, tag="rstdg")
                    nc.vector.reciprocal(rstdg[:], stdg[:])
                    nmrg = lns.tile([P, GRP], F32, tag="nmrg")
                    nc.vector.tensor_mul(nmrg[:], mvg[:, :, 0], rstdg[:])
                    nc.vector.tensor_scalar_mul(nmrg[:], nmrg[:], -1.0)
                    for j in range(GRP):
                        ht = hfp.tile([P, C], F8, tag="ht")
                        if j % 2:
                            nc.scalar.activation(ht[:], xts[g2 * GRP + j][:],
                                                 ACT.Identity,
                                                 bias=nmrg[:, j:j + 1],
                                                 scale=rstdg[:, j:j + 1])
                        else:
                            nc.vector.tensor_scalar(
                                ht[:], xts[g2 * GRP + j][:],
                                rstdg[:, j:j + 1], nmrg[:, j:j + 1],
                                op0=ALU.mult, op1=ALU.add)
                        nc.gpsimd.dma_start(
                            h_blks[gp][:, (g2 * GRP + j) * P:
                                       (g2 * GRP + j + 1) * P, :]
                            .rearrange("c t f -> t c f"),
                            ht[:].bitcast(BF16)
                            .rearrange("p (c f) -> p c f", f=P))
                # --- batched transposes, sync queue only ---
                h_t = htp.tile([P, KP, 1024], BF16, tag="h_t")
                for ko in range(KP):
                    nc.sync.dma_start_transpose(h_t[:, ko, :],
                                                h_blks[gp][ko, :, :])
                # v transposes of the previous pair (after this pair's h
                # transposes so they don't block the next QKV round)
                if gp > 0:
                    for t8 in range(8):
                        nc.sync.dma_start_transpose(
                            v_sb[:, (gp - 1) * 8 + t8, :],
                            v_blks[gp - 1][t8])
                # --- prefetch next pair's x before matmul-dependent stores
                if gp + 1 < NPAIR:
                    nxts = load_pair(lnp, gp + 1)
                # fp8 pair view: [p][ko][two][tok] over 1024 tokens
                hp = (h_t[:].bitcast(F8)
                      .rearrange("p c (t two) -> p c two t", two=2))
                # --- QKV DoubleRow matmuls (two 512-token halves) ---
                v_td = vtp.tile([P, HPC, 1024], BF16, tag="v_td")
                for g2 in range(2):
                    g = 2 * gp + g2
                    hsl = hp[:, :, :, g2 * 512:(g2 + 1) * 512]
                    for fc in range(4):  # q0,q1,k0,k1 feature chunks
                        ps = qps.tile([P, 512], F32, tag="qk_ps")
                        for ko in range(KP):
                            nc.tensor.matmul(
                                ps[:], w_qk_sb[:, ko, :, fc * P:(fc + 1) * P],
                                hsl[:, ko],
                                start=(ko == 0), stop=(ko == KP - 1),
                                perf_mode=DR)
                        nc.scalar.activation(
                            qk_t[:, fc, g * 512:(g + 1) * 512], ps[:],
                            ACT.Identity, bias=b_qk_sb[:, fc:fc + 1],
                            scale=inv_ws)
                    for fc in range(HPC):  # v feature-major via DR
                        psv = vps.tile([P, 512], F32, tag="v_ps")
                        for ko in range(KP):
                            nc.tensor.matmul(
                                psv[:], w_v_sb[:, ko, :, fc * P:(fc + 1) * P],
                                hsl[:, ko],
                                start=(ko == 0), stop=(ko == KP - 1),
                                perf_mode=DR)
                        nc.scalar.activation(
                            v_td[:, fc, g2 * 512:(g2 + 1) * 512], psv[:],
                            ACT.Identity, bias=b_v_sb[:, fc:fc + 1],
                            scale=inv_ws)
                # v blocked store: [t8][fc*128+p][tok]
                for fc in range(HPC):
                    nc.gpsimd.dma_start(
                        v_blks[gp][:, fc * P:(fc + 1) * P, :]
                        .rearrange("t p k -> p t k"),
                        v_td[:, fc, :].rearrange("p (t k) -> p t k", k=P))
                if gp + 1 < NPAIR:
                    xts = nxts
            for t8 in range(8):
                nc.sync.dma_start_transpose(
                    v_sb[:, (NPAIR - 1) * 8 + t8, :],
                    v_blks[NPAIR - 1][t8])

        if "C" not in ph:  # dump q_t head 0 into yt and stop
            with tc.tile_pool(name="dmp", bufs=2) as dmp:
                for rb in range(2):
                    t = dmp.tile([P, NTOK], F8, tag="t")
                    nc.vector.tensor_copy(t[:], qk_t[:, rb, :])
                    nc.sync.dma_start(yt_out[rb * P:(rb + 1) * P, :], t[:])
            return

        # ---------------- Phase C: causal attention ----------------
        with tc.tile_pool(name="sps", bufs=2, space="PSUM") as sps, \
             tc.tile_pool(name="yps", bufs=2, space="PSUM") as yps, \
             tc.tile_pool(name="rps", bufs=2, space="PSUM") as rps, \
             tc.tile_pool(name="attp", bufs=2) as attp, \
             tc.tile_pool(name="rvp", bufs=2) as rvp, \
             tc.tile_pool(name="yfp", bufs=3) as yfp:
            inv_sqrt_hd = 1.0 / math.sqrt(HD)
            for b in range(B):
                for hl in range(HPC):
                    q_sl = qk_t[:, hl, b * T:(b + 1) * T]
                    k_sl = qk_t[:, 2 + hl, b * T:(b + 1) * T]
                    for jt in range(JTT):
                        nblk = 4 * (jt + 1)
                        pt = attp.tile([P, 16, 512], BF16, tag="pt")
                        y_ps = yps.tile([P, 512], F32, tag="y_ps")
                        # all-ones stationary: every partition of rs_ps
                        # receives the rowsum (broadcast inside the matmul)
                        rs_ps = rps.tile([P, 512], F32, tag="rs_ps")
                        for ib2 in range(nblk // 2):
                            sp = sps.tile([P, 1024], F32, tag="s_ps")
                            c0s = []
                            for u in range(2):
                                ib = 2 * ib2 + u
                                c0 = max(0, ib * P - jt * 512)
                                c0s.append(c0)
                                nc.tensor.matmul(
                                    sp[:, u * 512 + c0:(u + 1) * 512],
                                    k_sl[:, ib * P:(ib + 1) * P],
                                    q_sl[:, jt * 512 + c0:(jt + 1) * 512],
                                    start=True, stop=True)
                            c0a = c0s[0]
                            pt2 = pt[:, 2 * ib2:2 * ib2 + 2, :].rearrange(
                                "p a b -> p (a b)")
                            nc.scalar.activation(
                                pt2[:, c0a:1024], sp[:, c0a:1024],
                                ACT.Exp, scale=inv_sqrt_hd)
                            for u in range(2):
                                ib = 2 * ib2 + u
                                c0 = c0s[u]
                                if ib >= 4 * jt:  # diagonal 128x128 sub-block
                                    nc.vector.tensor_mul(
                                        pt[:, ib, c0:c0 + P],
                                        pt[:, ib, c0:c0 + P], mask_sb[:])
                                vv = v_sb[:, b * (T // P) + ib,
                                          hl * HD:(hl + 1) * HD]
                                nc.tensor.matmul(
                                    y_ps[:, c0:512], vv, pt[:, ib, c0:512],
                                    start=(ib == 0), stop=(ib == nblk - 1))
                                nc.tensor.matmul(
                                    rs_ps[:, c0:512], ones_sb[:],
                                    pt[:, ib, c0:512],
                                    start=(ib == 0), stop=(ib == nblk - 1))
                        rinv = rvp.tile([P, 512], F32, tag="rinv")
                        nc.vector.reciprocal_approx_fast(rinv[:], rs_ps[:])
                        yf = yfp.tile([P, 512], F8, tag="yf")
                        nc.vector.tensor_mul(yf[:], y_ps[:], rinv[:])
                        nc.gpsimd.dma_start(
                            yt_out[hl * HD:(hl + 1) * HD,
                                   b * T + jt * 512:b * T + (jt + 1) * 512],
                            yf[:])


def _build_attn():
    nc = bacc.Bacc("TRN2", target_bir_lowering=False, debug=False,
                   num_devices=N_CORES, num_swdge_queues=4)
    io = {}
    io["x_full"] = nc.dram_tensor("x_full", [NTOK, C], F32,
                                  kind="ExternalInput").ap()
    io["w_qk"] = nc.dram_tensor("w_qk", [KP, P, 2, 4 * P], F8,
                                kind="ExternalInput")
    io["b_qk"] = nc.dram_tensor("b_qk", [4 * P], F32, kind="ExternalInput")
    io["w_v"] = nc.dram_tensor("w_v", [KP, P, 2, HPC * HD], F8,
                               kind="ExternalInput")
    io["b_v"] = nc.dram_tensor("b_v", [HPC * HD], F32, kind="ExternalInput")
    io["cmask"] = nc.dram_tensor("cmask", [P, P], BF16,
                                 kind="ExternalInput").ap()
    io["yt"] = nc.dram_tensor("yt", [HPC * HD, NTOK], F8,
                              kind="ExternalOutput").ap()
    with tile.TileContext(nc) as tc:
        _emit_attn(nc, tc, io)
    nc.compile()
    return nc


# ======================= Launch 2: proj + MLP =======================

def _emit_mlp(nc, tc, io):
    y_t_in, x_rows = io["y_t"], io["x_rows"]
    w_pr, b_pr = io["w_pr"], io["b_pr"]
    w_fc, b_fc, w_out, b_out = io["w_fc"], io["b_fc"], io["w_out"], io["b_out"]
    out = io["out"]
    ph = _PHASES
    inv_ws = 1.0 / WS

    from contextlib import ExitStack

    with ExitStack() as es:
        constp = es.enter_context(tc.tile_pool(name="constp", bufs=1))
        dramp = es.enter_context(tc.tile_pool(name="dramp", bufs=1,
                                              space="DRAM"))
        b_fc_sb = constp.tile([P, MC4], F32, name="b_fc_sb")
        nc.sync.dma_start(b_fc_sb[:], b_fc.ap().rearrange("(c p) -> p c", p=P))
        b_pr_sb = constp.tile([P, C], F32, name="b_pr_sb")
        nc.scalar.dma_start(b_pr_sb[:], b_pr.ap()[None, :].to_broadcast((P, C)))
        b_out_sb = constp.tile([P, C], F32, name="b_out_sb")
        nc.scalar.dma_start(b_out_sb[:], b_out.ap()[None, :].to_broadcast((P, C)))

        # blocked x2 storage: KC chunks of [512, 128] bf16
        x2_blk = dramp.tile([KC, 512, P], BF16, name="x2_blk")

        persde = es.enter_context(tc.tile_pool(name="persde", bufs=1))
        x2_f32 = persde.tile([P, 4, C], F32, name="x2_f32")
        x2_t = persde.tile([P, KC, RPC], BF16, name="x2_t")
        y_sb = persde.tile([P, KP, 2, RPC], F8, name="y_sb")

        # ---------------- Phase D: proj + residual ----------------
        with tc.tile_pool(name="pdp", bufs=3) as pdp, \
             tc.tile_pool(name="wprp", bufs=4) as wprp, \
             tc.tile_pool(name="dps", bufs=3, space="PSUM") as dps:
            y_re = y_t_in.ap().rearrange("(ko i p) r -> p ko i r", p=P, i=2)
            for ko in range(KP):
                nc.sync.dma_start(y_sb[:, ko], y_re[:, ko])
            for rb in range(4):
                nc.scalar.dma_start(
                    x2_f32[:, rb, :], x_rows[rb * P:(rb + 1) * P, :])
            for rb in range(4):
                nc.vector.tensor_add(
                    x2_f32[:, rb, :], x2_f32[:, rb, :], b_pr_sb[:])
            wts = []
            for ct in range(4):
                wt = wprp.tile([P, KP, 2, 512], F8, tag="wpr")
                nc.sync.dma_start(
                    wt[:], w_pr.ap()[ct].rearrange("ko p two f -> p ko two f"))
                wts.append(wt)
            for ct in range(4):
                wt = wts[ct]
                for rb in range(4):
                    ps = dps.tile([P, 512], F32, tag="pr_ps")
                    for ko in range(KP):
                        nc.tensor.matmul(
                            ps[:], y_sb[:, ko, :, rb * P:(rb + 1) * P],
                            wt[:, ko],
                            start=(ko == 0), stop=(ko == KP - 1),
                            perf_mode=DR)
                    sl = x2_f32[:, rb, ct * 512:(ct + 1) * 512]
                    nc.vector.scalar_tensor_tensor(
                        sl, ps[:], inv_ws, sl, op0=ALU.mult, op1=ALU.add)
                    x2b = pdp.tile([P, 512], BF16, tag="x2b")
                    nc.vector.tensor_copy(x2b[:], sl)
                    nc.gpsimd.dma_start(
                        x2_blk[ct * 4:(ct + 1) * 4, rb * P:(rb + 1) * P, :]
                        .rearrange("c t f -> t c f"),
                        x2b[:].rearrange("p (c f) -> p c f", f=P))
                # transpose-load this ct's feature chunks (pairs: [1024,128])
                for kk in range(2):
                    nc.sync.dma_start_transpose(
                        x2_t[:, 4 * ct + 2 * kk:4 * ct + 2 * kk + 2, :]
                        .rearrange("p c t -> p (c t)"),
                        x2_blk[4 * ct + 2 * kk:4 * ct + 2 * kk + 2, :, :]
                        .rearrange("c t f -> (c t) f"))

        if "E" not in ph:  # dump x2 and stop
            with tc.tile_pool(name="dmp", bufs=2) as dmp:
                for rb in range(4):
                    nc.sync.dma_start(
                        out[rb * P:(rb + 1) * P, :], x2_f32[:, rb, :])
            return

        # pre-add b_out into the residual (after proj phase)
        for rb in range(4):
            nc.vector.tensor_add(
                x2_f32[:, rb, :], x2_f32[:, rb, :], b_out_sb[:])

        # ---------------- Phase E: MLP + residual ----------------
        with tc.tile_pool(name="mep", bufs=1) as mep, \
             tc.tile_pool(name="wfcp", bufs=4) as wfcp, \
             tc.tile_pool(name="wop", bufs=8) as wop, \
             tc.tile_pool(name="ofp", bufs=3) as ofp, \
             tc.tile_pool(name="eps1", bufs=3, space="PSUM") as eps1, \
             tc.tile_pool(name="eps2", bufs=1, space="PSUM") as eps2:
            m_sb = mep.tile([P, MC4, RPC], BF16, name="m_sb")
            for mc in range(MC4):
                wfc_t = wfcp.tile([P, KC, P], BF16, tag="wfc")
                nc.sync.dma_start(
                    wfc_t[:],
                    w_fc.ap()[:, mc * P:(mc + 1) * P]
                    .rearrange("(ko p) m -> p ko m", p=P))
                ps = eps1.tile([P, 512], F32, tag="fc_ps")
                for ko in range(KC):
                    nc.tensor.matmul(
                        ps[:], wfc_t[:, ko, :], x2_t[:, ko, :],
                        start=(ko == 0), stop=(ko == KC - 1))
                nc.scalar.activation(
                    m_sb[:, mc, :], ps[:], ACT.Gelu,
                    bias=b_fc_sb[:, mc:mc + 1], scale=1.0)
            if "F" in ph:  # debug: dump m chunks 0..15 feature-major
                with tc.tile_pool(name="dmp2", bufs=2) as dmp2:
                    for mc in range(16):
                        mf = dmp2.tile([P, RPC], F32, tag="mf")
                        nc.vector.tensor_copy(mf[:], m_sb[:, mc, :])
                        nc.sync.dma_start(
                            out[(mc % 4) * P:(mc % 4 + 1) * P,
                                (mc // 4) * 512:(mc // 4 + 1) * 512],
                            mf[:])
                return
            for ct in range(4):
                pss = [eps2.tile([P, 512], F32, tag=f"o_ps{rb}",
                                 name=f"o_ps{rb}_{ct}")
                       for rb in range(4)]
                for ko in range(MC4):
                    wo_t = wop.tile([P, 512], BF16, tag="wo")
                    nc.sync.dma_start(
                        wo_t[:],
                        w_out.ap()[ko * P:(ko + 1) * P,
                                   ct * 512:(ct + 1) * 512])
                    for rb in range(4):
                        nc.tensor.matmul(
                            pss[rb][:], m_sb[:, ko, rb * P:(rb + 1) * P],
                            wo_t[:],
                            start=(ko == 0), stop=(ko == MC4 - 1))
                for rb in range(4):
                    of = ofp.tile([P, 512], F32, tag="of")
                    nc.vector.tensor_add(
                        of[:], pss[rb][:],
                        x2_f32[:, rb, ct * 512:(ct + 1) * 512])
                    nc.scalar.dma_start(
                        out[rb * P:(rb + 1) * P, ct * 512:(ct + 1) * 512],
                        of[:])


def _build_mlp():
    nc = bacc.Bacc("TRN2", target_bir_lowering=False, debug=False,
                   num_devices=N_CORES)
    io = {}
    io["y_t"] = nc.dram_tensor("y_t", [C, RPC], F8, kind="ExternalInput")
    io["x_rows"] = nc.dram_tensor("x_rows", [RPC, C], F32,
                                  kind="ExternalInput").ap()
    io["w_pr"] = nc.dram_tensor("w_pr", [4, KP, P, 2, 512], F8,
                                kind="ExternalInput")
    io["b_pr"] = nc.dram_tensor("b_pr", [C], F32, kind="ExternalInput")
    io["w_fc"] = nc.dram_tensor("w_fc", [C, F4], BF16, kind="ExternalInput")
    io["b_fc"] = nc.dram_tensor("b_fc", [F4], F32, kind="ExternalInput")
    io["w_out"] = nc.dram_tensor("w_out", [F4, C], BF16, kind="ExternalInput")
    io["b_out"] = nc.dram_tensor("b_out", [C], F32, kind="ExternalInput")
    io["out"] = nc.dram_tensor("out", [RPC, C], F32,
                               kind="ExternalOutput").ap()
    with tile.TileContext(nc) as tc:
        _emit_mlp(nc, tc, io)
    nc.compile()
    return nc


def _get_built():
    global _BUILT1, _BUILT2
    if _BUILT1 is None:
        _BUILT1 = _build_attn()
    if _BUILT2 is None and any(p in _PHASES for p in "DE"):
        _BUILT2 = _build_mlp()
    return _BUILT1, _BUILT2


# ======================= Host orchestration =======================

def _pack_pairs_interleaved(w, scale):
    """[C, F] -> [KP, 128, 2, F] fp8, pair rows (256k+2j, 256k+2j+1)."""
    f8 = ml_dtypes.float8_e4m3
    wf = np.asarray(w, np.float64) * scale
    Cd, F = wf.shape
    wr = wf.reshape(Cd // 256, 128, 2, F)  # [ko][j][i] = row 256ko+2j+i
    return np.ascontiguousarray(wr.astype(np.float32)).astype(f8)


def _pack_pairs_stacked(w, scale):
    """[C, F] -> [KP, 128, 2, F] fp8, pair rows (256k+j, 256k+128+j)."""
    f8 = ml_dtypes.float8_e4m3
    wf = np.asarray(w, np.float64) * scale
    Cd, F = wf.shape
    wr = wf.reshape(Cd // 256, 2, 128, F).transpose(0, 2, 1, 3)
    return np.ascontiguousarray(wr.astype(np.float32)).astype(f8)


def _prep(x, ln_scale, ln_bias, w_qkv, b_qkv, w_proj, b_proj,
          w_fc, b_fc, w_out, b_out):
    bf = ml_dtypes.bfloat16
    xf = np.ascontiguousarray(np.asarray(x, np.float32).reshape(NTOK, C))
    # Fold LN affine into the QKV projection (exact, in float64).
    w64 = np.asarray(w_qkv, np.float64)
    g = np.asarray(ln_scale, np.float64)
    beta = np.asarray(ln_bias, np.float64)
    w_eff = g[:, None] * w64
    b_eff = np.asarray(b_qkv, np.float64) + beta @ w64

    wq, wk, wv = w_eff[:, :C], w_eff[:, C:2 * C], w_eff[:, 2 * C:]
    bq, bk, bv = b_eff[:C], b_eff[C:2 * C], b_eff[2 * C:]
    cmask = np.triu(np.ones((P, P), np.float32)).astype(bf)

    in1 = []
    for i in range(N_CORES):
        hs = slice(i * HPC * HD, (i + 1) * HPC * HD)
        w_qk_i = _pack_pairs_interleaved(
            np.concatenate([wq[:, hs], wk[:, hs]], axis=1), WS)
        b_qk_i = np.ascontiguousarray(
            np.concatenate([bq[hs], bk[hs]]).astype(np.float32))
        w_v_i = _pack_pairs_interleaved(wv[:, hs], WS)
        b_v_i = np.ascontiguousarray(bv[hs].astype(np.float32))
        in1.append({
            "x_full": xf,
            "w_qk": w_qk_i, "b_qk": b_qk_i, "w_v": w_v_i, "b_v": b_v_i,
            "cmask": cmask,
        })

    # proj weights: [4ct][KP, 128, 2, 512] fp8, stacked pairing
    wp = _pack_pairs_stacked(np.asarray(w_proj, np.float64), WS)  # [8,128,2,2048]
    w_pr_b = np.ascontiguousarray(
        wp.reshape(KP, P, 2, 4, 512).transpose(3, 0, 1, 2, 4))
    w_fc_b = np.asarray(w_fc, np.float32).astype(bf)
    w_out_b = np.asarray(w_out, np.float32).astype(bf)
    b_pr_f = np.ascontiguousarray(np.asarray(b_proj, np.float32))
    b_fc_f = np.ascontiguousarray(np.asarray(b_fc, np.float32))
    b_out_f = np.ascontiguousarray(np.asarray(b_out, np.float32))
    in2_common = {
        "w_pr": w_pr_b, "b_pr": b_pr_f, "w_fc": w_fc_b, "b_fc": b_fc_f,
        "w_out": w_out_b, "b_out": b_out_f,
    }
    return xf, in1, in2_common


def run(inputs, trace=False, trace_cores=None):
    """Run both SPMD launches. Returns (output [B,T,C] f32, res1, res2)."""
    nc1, nc2 = _get_built()
    xf, in1, in2_common = _prep(**inputs)
    kwargs = {}
    if trace:
        kwargs = dict(trace=True,
                      trace_cores=trace_cores if trace_cores else [0])
    res1 = run_bass_kernel_spmd(nc1, in1, core_ids=list(range(N_CORES)),
                                **kwargs)
    y_all = np.concatenate(
        [np.asarray(res1.results[i]["yt"]) for i in range(N_CORES)], axis=0)
    if nc2 is None:
        return y_all, res1, None

    in2 = []
    for i in range(N_CORES):
        in2.append({
            "y_t": np.ascontiguousarray(y_all[:, i * RPC:(i + 1) * RPC]),
            "x_rows": np.ascontiguousarray(xf[i * RPC:(i + 1) * RPC]),
            **in2_common,
        })
    res2 = run_bass_kernel_spmd(nc2, in2, core_ids=list(range(N_CORES)),
                                **kwargs)
    outf = np.concatenate(
        [np.asarray(res2.results[i]["out"]) for i in range(N_CORES)], axis=0)
    return outf.reshape(B, T, C).astype(np.float32), res1, res2


def kernel(**inputs):
    out, _, _ = run(inputs, trace=False)
    return out
